# revision 82
# baseline (speedup 1.0000x reference)
"""Multi-head attention (B=2, N=2048, DIM=1024, H=16) on 8 Trainium2 NeuronCores.

Sharding: tensor-parallel by head within two quads (cores 0-3 -> batch 0,
cores 4-7 -> batch 1; quad rank r owns heads 4r..4r+3). Each core computes
Q/K/V projections for its 4 heads and masked-softmax attention. The output
projection is sharded over tokens ACROSS BATCHES: core c owns the 128-token
block [1024*nh + 128*c, +128) of BOTH batches for each query chunk nh, so
the 8-core head->token AllToAll carries no wasted bytes and splits into two
chunk collectives; the first overlaps the second chunk's attention compute.

Key engine assignment (vs the identity-matmul baseline):
- mask is applied multiplicatively on the vector engine (pm = exp(s)*mask01,
  both multiplies on DVE - the pool engine is 2x slower and its longer SBUF
  occupancy throttles the PE stream rate: ~620ns vs ~437ns per 512-col
  matmul), removing ~100us/core of PE identity-matmul time;
- the attn@V matmuls for tile t-1 are emitted after the score matmuls for
  tile t (software pipelining), so the in-order tensor queue never stalls
  on exp/mask of the current tile;
- softmax denominators come from a ones-column in V (row 64 of the attn@V
  accumulator) and ride RAW through the AllToAll as row 64 of each head
  block; one batched reciprocal per chunk + a PE broadcast (host-provided
  0/1 pattern, partition-aligned) normalizes after the collective;
- the scalar engine runs ONLY the exp activations (~140us/core); PSUM
  evictions run on the vector engine; phase-3 input DMAs are issued from
  both the sync and pool queues and are emitted right after each
  collective so they fire mid-attention.

Numerics: matmuls bf16 with fp32 PSUM accumulation (fp8 was tested and is
numerically dead here: query-side quantization noise is correlated across
keys, giving ~4% output error vs the 2% gate); exp without max-subtraction
(scores ~N(0,1) after scaling); denominators in bf16 (~0.4% relerr).
NOTE: the mask multiply must NOT be done in place (pu *= mask): the DVE
read-modify-write showed rare timing-dependent corruption (~7% of profiled
runs produced 7e-2 rel err); the separate pm tile is deliberate.
Measured end-to-end L2 relative error ~6.7e-3; HW exec ~350-385us
(baseline: 535us; run-to-run variance is DVFS throttling).
"""

import numpy as np
import ml_dtypes

import concourse.bass as bass
import concourse.mybir as mybir
import concourse.tile as tile

F32 = mybir.dt.float32
BF16 = mybir.dt.bfloat16
BF16_NP = ml_dtypes.bfloat16

B, DIM, H = 2, 1024, 16
N_FULL = 2048
HD = DIM // H          # 64
SCALE = HD ** -0.5     # 0.125
NCORES = 8
H_LOC = H // 4         # 4 heads per core
COLS = H_LOC * HD      # 256 local channels
KT_D = DIM // 128      # 8 contraction tiles over DIM
GROUPS = [list(range(NCORES))]


# ---------------------------------------------------------------------------
# Workaround: this walrus build rejects >2 sync waits on one instruction
# ("Too many sync wait commands" in setupSyncWait). The TileContext final
# drain aggregates one wait per logical processor; split it into a chain of
# single-wait drains.
# ---------------------------------------------------------------------------
def _patch_tile_drain():
    from bass_rust import ScopedClock

    if getattr(tile.TileContext, "_drain_patched", False):
        return

    def _drain_and_barrier(self, tick_clock, wait_clock):
        nc = self.nc
        drain_inst = nc.sync.drain()
        wait_clock.add_sem_waits(
            drain_inst.ins, ScopedClock({None: tick_clock.global_clock})
        )
        si = drain_inst.ins.sync_info
        if si is not None and len(si.on_wait) > 1:
            waits = list(si.on_wait)
            drain_inst.ins.sync_info = mybir.SyncInfo(
                on_wait=waits[:1], on_update=list(si.on_update)
            )
            for w in waits[1:]:
                d = nc.sync.drain()
                dsi = d.ins.sync_info
                upd = list(dsi.on_update) if dsi is not None else []
                d.ins.sync_info = mybir.SyncInfo(on_wait=[w], on_update=upd)

        nc.all_engine_barrier()
        assert self.sems is not None
        popped = nc._tile_sem_poison_stack.pop()
        assert popped is self._sem_poison
        nc.clear_and_free_semaphores(list(self.sems.allocated().values()))
        nc.all_engine_barrier()

    tile.TileContext._drain_and_barrier = _drain_and_barrier
    tile.TileContext._drain_patched = True


def _split_sync_waits(nc, maxw=1):
    """Walrus in this build rejects instructions carrying more than a couple
    of semaphore waits. Move excess waits onto injected same-engine NoOps
    immediately before the instruction (identical semantics: the engine
    blocks at the nop instead of at the instruction itself)."""
    n_split = 0
    for f in nc.m.functions:
        for bb in f.blocks:
            new_insts = []
            for ins in bb.instructions:
                si = ins.sync_info
                if si is not None and len(si.on_wait) > maxw:
                    waits = list(si.on_wait)
                    for i, w in enumerate(waits[maxw:]):
                        nop = mybir.InstNoOp(
                            name=f"{ins.name}-w{i}", ins=[], outs=[]
                        )
                        nop.engine = ins.engine
                        nop.sync_info = mybir.SyncInfo(
                            on_wait=[w], on_update=[]
                        )
                        new_insts.append(nop)
                    ins.sync_info = mybir.SyncInfo(
                        on_wait=waits[:maxw], on_update=list(si.on_update)
                    )
                    n_split += 1
                new_insts.append(ins)
            bb.instructions = new_insts
    return n_split


def build_nc(N=N_FULL, split_waits=True):
    """Build the per-core Bass program (same SPMD program for all 8 cores)."""
    _patch_tile_drain()
    assert N % 1024 == 0
    NSLICE = N // 4            # output rows per core
    MT = N // 128              # m-tiles over keys
    HS = 1024                  # attention query-chunk size
    NH = N // HS               # number of query chunks
    CT = DIM // 128            # contraction tiles for output projection

    nc = bass.Bass(trn_type="TRN2", num_devices=NCORES)

    xqT_e = nc.declare_dram_parameter("xqT", [DIM, N], BF16, isOutput=False)
    xkT_e = nc.declare_dram_parameter("xkT", [DIM, N], BF16, isOutput=False)
    xvT_e = nc.declare_dram_parameter("xvT", [DIM, N], BF16, isOutput=False)
    wq_e = nc.declare_dram_parameter("wq", [DIM, COLS], BF16, isOutput=False)
    wk_e = nc.declare_dram_parameter("wk", [DIM, COLS], BF16, isOutput=False)
    wv_e = nc.declare_dram_parameter("wv", [DIM, COLS], BF16, isOutput=False)
    wp_e = nc.declare_dram_parameter("wp", [DIM, DIM], BF16, isOutput=False)
    maskT_e = nc.declare_dram_parameter("maskT", [N, N], BF16, isOutput=False)
    bpr_e = nc.declare_dram_parameter("bp_rep", [128, DIM], F32, isOutput=False)
    pb_e = nc.declare_dram_parameter("pb_all", [64, CT * 128], BF16,
                                     isOutput=False)
    out_e = nc.declare_dram_parameter("out", [NSLICE, DIM], F32, isOutput=True)

    # per-chunk AllToAll buffers: chunk to dest j = [65 rows, (head g,
    # 128 tokens)]; row 64 carries the raw denominators. 1KB rows keep the
    # collective's DMA descriptors efficient.
    a2a_in = [nc.dram_tensor(f"a2a_in{i}", [NCORES * 65, H_LOC * 128], BF16)
              for i in range(NH)]
    a2a_out = [nc.dram_tensor(f"a2a_out{i}", [NCORES * 65, H_LOC * 128], BF16)
               for i in range(NH)]

    with tile.TileContext(nc) as tc:
        with (
            tc.tile_pool(name="cpool", bufs=1) as cpool,
            tc.tile_pool(name="xstream", bufs=6) as xpool,
            tc.tile_pool(name="mpool", bufs=18) as mpool,
            tc.tile_pool(name="pupool", bufs=3) as pupool,
            tc.tile_pool(name="pmpool", bufs=6) as pmpool,
            tc.tile_pool(name="p3pool", bufs=3) as p3pool,
            tc.tile_pool(name="opool", bufs=2) as opool,
            tc.tile_pool(name="ps", bufs=1, space="PSUM") as ps,
        ):
            # PSUM: four 2-bank (4KB/partition) tag slots shared by all
            # phases; static pool allocation = 8 banks.
            PST = ["S0", "S1", "V0", "V1"]

            # ---- long-lived SBUF tensors -------------------------------
            qt_sb = [cpool.tile([128, N], BF16, tag=f"qt{i}", name=f"qt{i}")
                     for i in range(2)]
            kt_sb = [cpool.tile([128, N], BF16, tag=f"kt{i}", name=f"kt{i}")
                     for i in range(2)]
            # V per m-tile: [m, head, 65]; cols 0..63 = V_head, col 64 = ones
            vt_sb = [cpool.tile([128, H_LOC, 65], BF16, tag=f"vt{t}",
                                name=f"vt{t}")
                     for t in range(MT)]
            # per-local-head x^T (unnormalized), row 64 = 1/denominator
            xt_sb = cpool.tile([65, H_LOC, N], BF16, tag="xth", name="xth")
            bpr_sb = cpool.tile([128, DIM], F32, tag="bpr", name="bpr")
            wq_sb = cpool.tile([128, KT_D, COLS], BF16, tag="wq", name="wq")
            wk_sb = cpool.tile([128, KT_D, COLS], BF16, tag="wk", name="wk")
            wv_sb = cpool.tile([128, KT_D, COLS], BF16, tag="wv", name="wv")
            xv_sb = cpool.tile([128, KT_D, N], BF16, tag="xv", name="xv")
            wp_sb = cpool.tile([128, CT, DIM], BF16, tag="wp", name="wp")
            # phase-3 denominator broadcast patterns: for batch section bs
            # at base partition 32*bs, row (32*bs + h16), slab ct, col p is
            # 1 iff h16 == 2*ct + p//64
            pb_sb = cpool.tile([64, CT, 128], BF16, tag="pb", name="pb")
            # gathered raw denominators, rows (32*bs + 4*src + g); unused
            # rows stay at the memset value
            rc_sb = [cpool.tile([64, 128], BF16, tag=f"rc{i}", name=f"rc{i}")
                     for i in range(NH)]

            # ---- phase 1: projections ----------------------------------
            # Q^T and K^T: [COLS, N] as two 128-row blocks; kt-outer with
            # one live [128, 1024] psum accumulator per (block, n-half).
            # DMA emission order keeps the startup critical path minimal:
            # only wq + the first xq tiles gate the first matmuls; all
            # later-phase constants are emitted after the Q loop starts.
            wq_v = wq_e[:].rearrange("(kt p) c -> p kt c", p=128)
            wk_v = wk_e[:].rearrange("(kt p) c -> p kt c", p=128)
            wv_v = wv_e[:].rearrange("(kt p) c -> p kt c", p=128)
            nc.sync.dma_start(wq_sb[:, 0:2, :], wq_v[:, 0:2, :])
            nc.sync.dma_start(wq_sb[:, 2:KT_D, :], wq_v[:, 2:KT_D, :])
            NH2 = N // 1024
            for w_sb, x_e, dst in (
                (wq_sb, xqT_e, qt_sb),
                (wk_sb, xkT_e, kt_sb),
            ):
                psums = [ps.tile([128, 1024], F32, tag=PST[cb * NH2 + n2],
                                 name="p1qk")
                         for cb in range(2) for n2 in range(NH2)]
                for kt in range(KT_D):
                    xt_t = xpool.tile([128, N], BF16, tag="xs", name="xs")
                    # two half-tile DMAs so the first matmuls start sooner
                    nc.sync.dma_start(
                        xt_t[:, 0:N // 2],
                        x_e[128 * kt:128 * (kt + 1), 0:N // 2],
                    )
                    nc.sync.dma_start(
                        xt_t[:, N // 2:N],
                        x_e[128 * kt:128 * (kt + 1), N // 2:N],
                    )
                    for cb in range(2):
                        for nch in range(N // 512):
                            n2, ch = divmod(nch, 2)
                            nc.tensor.matmul(
                                psums[cb * NH2 + n2][:, 512 * ch:512 * (ch + 1)],
                                w_sb[:, kt, 128 * cb:128 * (cb + 1)],
                                xt_t[:, 512 * nch:512 * (nch + 1)],
                                start=(kt == 0), stop=(kt == KT_D - 1),
                            )
                    if w_sb is wq_sb:
                        if kt == 0:
                            nc.sync.dma_start(wk_sb[:], wk_v)
                        elif kt == 1:
                            nc.sync.dma_start(wv_sb[:], wv_v)
                    else:
                        # stream xv in per-kt slices through the K loop so
                        # it never lumps against the x-tile stream
                        xv_v = xvT_e[:].rearrange("(kt p) n -> p kt n", p=128)
                        nc.sync.dma_start(xv_sb[:, kt, :], xv_v[:, kt, :])
                with nc.allow_low_precision(reason="bf16 Q/K"):
                    for cb in range(2):
                        for n2 in range(NH2):
                            nc.vector.tensor_copy(
                                dst[cb][:, 1024 * n2:1024 * (n2 + 1)],
                                psums[cb * NH2 + n2][:],
                            )

            # late-phase constants (emitted once the QK streams are rolling)
            nc.sync.dma_start(bpr_sb[:], bpr_e[:])
            nc.sync.dma_start(
                pb_sb[:], pb_e[:].rearrange("r (ct i) -> r ct i", ct=CT)
            )
            for i in range(NH):
                nc.gpsimd.memset(rc_sb[i][:], 1.0)
            for t in range(MT):
                nc.gpsimd.memset(vt_sb[t][:, :, 64:65], 1.0)

            # V in natural layout: out[m-tile, 4*HD] = xvT_kt^T @ wv_kt
            for t in range(MT):
                vps = ps.tile([128, 1024], F32, tag=PST[t % 2], name="p1v")
                for kt in range(KT_D):
                    nc.tensor.matmul(
                        vps[:, 0:COLS],
                        xv_sb[:, kt, 128 * t:128 * (t + 1)],
                        wv_sb[:, kt, :],
                        start=(kt == 0), stop=(kt == KT_D - 1),
                    )
                with nc.allow_low_precision(reason="bf16 V"):
                    nc.vector.tensor_copy(
                        vt_sb[t][:, :, 0:HD],
                        vps[:, 0:COLS].rearrange("p (h d) -> p h d", h=H_LOC),
                    )

            # ---- phase 2: attention ------------------------------------
            # Scores s = K_h^T Q_h per 128-key m-tile (head pairs packed in
            # 64-row PE groups); exp on the scalar engine; 0/1 mask applied
            # multiplicatively on the vector engine; attn@[V|ones]
            # accumulates x^T and the denominator row in PSUM.
            mask_t = {}
            p3_tiles = {}

            def load_mask(nh_, t_):
                if (nh_, t_) in mask_t:
                    return
                m = mpool.tile([128, HS], BF16, tag="m", name="m")
                nc.sync.dma_start(
                    m[:],
                    maskT_e[128 * t_:128 * (t_ + 1), HS * nh_:HS * (nh_ + 1)],
                )
                mask_t[(nh_, t_)] = m

            for t in range(4):
                load_mask(0, t)
            # wp is phase-3-only; emit its DMA after the mask prefetch
            wp_v = wp_e[:].rearrange("(ct p) c -> p ct c", p=128)
            nc.sync.dma_start(wp_sb[:], wp_v)

            for nh in range(NH):
                nsl = slice(HS * nh, HS * (nh + 1))
                for hp in range(2):
                    vo = [ps.tile([128, HS], F32, tag=PST[2 + h], name="vo")
                          for h in range(2)]

                    def av_mms(t_, pm_):
                        for h in range(2):
                            for ch in range(2):
                                csl = slice(512 * ch, 512 * (ch + 1))
                                nc.tensor.matmul(
                                    vo[h][0:65, csl],
                                    vt_sb[t_][:, 2 * hp + h, :],
                                    pm_[h][:, csl],
                                    start=(t_ == 0), stop=(t_ == MT - 1),
                                )

                    prev = None
                    for t in range(MT):
                        # prefetch masks: rest of this chunk during hp0,
                        # next chunk during hp1
                        if hp == 0 and t + 4 < MT:
                            load_mask(nh, t + 4)
                        elif hp == 1 and nh + 1 < NH:
                            load_mask(nh + 1, t)
                        s_ps = [ps.tile([128, HS], F32, tag=PST[h], name="s")
                                for h in range(2)]
                        for ch in range(2):
                            csl = slice(512 * ch, 512 * (ch + 1))
                            gsl = slice(HS * nh + 512 * ch,
                                        HS * nh + 512 * (ch + 1))
                            for h in range(2):
                                nc.tensor.matmul(
                                    s_ps[h][:, csl],
                                    kt_sb[hp][64 * h:64 * (h + 1),
                                              128 * t:128 * (t + 1)],
                                    qt_sb[hp][64 * h:64 * (h + 1), gsl],
                                    start=True, stop=True,
                                    tile_position=(64 * h, 0),
                                )
                        # software pipeline: the attn@V matmuls for t-1 are
                        # emitted AFTER the scores for t, so the tensor
                        # engine never waits on exp/mask of the current tile
                        if prev is not None:
                            av_mms(*prev)
                        pms = []
                        for h in range(2):
                            pu = pupool.tile([128, HS], BF16, tag="pu",
                                             name="pu")
                            nc.scalar.activation(
                                pu[:], s_ps[h][:],
                                mybir.ActivationFunctionType.Exp,
                                scale=float(SCALE),
                            )
                            pm = pmpool.tile([128, HS], BF16, tag="pm",
                                             name="pm")
                            with nc.allow_low_precision(reason="bf16 probs"):
                                nc.vector.tensor_mul(
                                    pm[:], pu[:], mask_t[(nh, t)][:]
                                )
                            pms.append(pm)
                        prev = (t, pms)
                    av_mms(*prev)
                    # evict x^T rows + raw denominator row (reciprocal
                    # happens post-AllToAll on small phase-3 tiles)
                    for h in range(2):
                        g = 2 * hp + h
                        with nc.allow_low_precision(reason="bf16 y/denom"):
                            nc.vector.tensor_copy(
                                xt_sb[0:64, g, nsl], vo[h][0:64, :]
                            )
                            nc.vector.tensor_copy(
                                xt_sb[64:65, g, nsl], vo[h][64:65, :]
                            )
                # AllToAll for this query chunk: head-split -> token-split
                # (dest core j owns tokens [1024*nh + 128*j, +128) of both
                # batches)
                a2a_in_v = a2a_in[nh][:].rearrange(
                    "(j p) c -> j p c", j=NCORES
                )
                for j in range(NCORES):
                    csl = slice(HS * nh + 128 * j, HS * nh + 128 * (j + 1))
                    nc.sync.dma_start(
                        a2a_in_v[j].rearrange("p (g i) -> p g i", g=H_LOC),
                        xt_sb[:, :, csl],
                    )
                nc.gpsimd.collective_compute(
                    "AllToAll",
                    mybir.AluOpType.bypass,
                    replica_groups=GROUPS,
                    ins=[a2a_in[nh][:]],
                    outs=[a2a_out[nh][:]],
                )
                # emit this chunk's phase-3 input DMAs NOW so the in-order
                # sync engine issues them as soon as the collective lands
                # (i.e. during the next chunk's attention compute)
                a2a_o_v = a2a_out[nh][:].rearrange(
                    "(s p) c -> s p c", s=NCORES
                )
                a2a_o_g = a2a_out[nh][:].rearrange(
                    "r (g i) -> (r g) i", g=H_LOC
                )
                # issue these from the (otherwise idle) pool engine queue
                # so they don't serialize behind the sync engine's DMAs
                for s in range(NCORES):
                    bs, sl = divmod(s, 4)
                    nc.gpsimd.dma_start(
                        rc_sb[nh][32 * bs + 4 * sl:32 * bs + 4 * sl + 4, :],
                        a2a_o_g[4 * (65 * s + 64):4 * (65 * s + 64) + 4, :],
                    )
                for ct in range(CT):
                    g0 = (2 * ct) % 4
                    for bs in range(2):
                        s = 4 * bs + (2 * ct) // 4
                        aa = p3pool.tile([128, 128], BF16, tag="aa",
                                         name="aa", bufs=24)
                        for i in range(2):
                            eng = nc.gpsimd if i == 0 else nc.sync
                            eng.dma_start(
                                aa[64 * i:64 * (i + 1), :],
                                a2a_o_v[s, 0:64,
                                        128 * (g0 + i):128 * (g0 + i + 1)],
                            )
                        p3_tiles[(nh, ct, bs)] = aa

            # ---- phase 3: normalize + output projection ----------------
            # Received layout: rows (src, 65), cols (g, 128 tok): head
            # 4*src+g has 64 x^T rows + a raw-denominator row. Two batch
            # sections (src 0-3 = batch 0, src 4-7 = batch 1); for each,
            # out[tok, :] = sum_ct (aa_ct / denom)^T @ Wp[ct] + bias.
            for nh in range(NH):
                # one batched reciprocal of all 32 gathered denominators
                rcf = p3pool.tile([64, 128], F32, tag="rcf", name="rcf")
                rcb = p3pool.tile([64, 128], BF16, tag="rcb", name="rcb")
                nc.vector.reciprocal(rcf[:], rc_sb[nh][:])
                with nc.allow_low_precision(reason="bf16 recip"):
                    nc.vector.tensor_copy(rcb[:], rcf[:])
                pj = [ps.tile([128, DIM], F32, tag=PST[bs], name=f"pj{bs}")
                      for bs in range(2)]
                for ct in range(CT):
                    for bs in range(2):
                        aa = p3_tiles[(nh, ct, bs)]
                        rr = ps.tile([128, 1024], F32, tag=PST[2 + bs],
                                     name="rr")
                        nc.tensor.matmul(
                            rr[:, 0:128],
                            pb_sb[32 * bs:32 * bs + 16, ct, :],
                            rcb[32 * bs:32 * bs + 16, :],
                            start=True, stop=True,
                        )
                        rrb = p3pool.tile([128, 128], BF16, tag="rrb",
                                          name="rrb")
                        xn = p3pool.tile([128, 128], BF16, tag="xn", name="xn")
                        with nc.allow_low_precision(reason="bf16 norm"):
                            nc.vector.tensor_copy(rrb[:], rr[:, 0:128])
                            nc.vector.tensor_mul(xn[:], aa[:], rrb[:])
                        for oc in range(2):
                            nc.tensor.matmul(
                                pj[bs][:, 512 * oc:512 * (oc + 1)],
                                xn[:],
                                wp_sb[:, ct, 512 * oc:512 * (oc + 1)],
                                start=(ct == 0), stop=(ct == CT - 1),
                            )
                for bs in range(2):
                    o_t = opool.tile([128, DIM], F32, tag="ot", name="ot")
                    nc.vector.tensor_add(o_t[:], pj[bs][:], bpr_sb[:])
                    blk = 2 * nh + bs
                    nc.sync.dma_start(
                        out_e[128 * blk:128 * (blk + 1), :], o_t[:]
                    )

    if split_waits:
        _split_sync_waits(nc)
    return nc


def make_in_maps(q, k, v, mask, Wq, Wk, Wv, Wp, bp, N=N_FULL):
    """Shard + pre-transpose + bf16-cast the full inputs for the 8 cores."""
    bf = lambda a: np.ascontiguousarray(a).astype(BF16_NP)
    bp_rep = np.ascontiguousarray(
        np.broadcast_to(bp.astype(np.float32), (128, DIM))
    )
    wp_bf = bf(Wp)
    in_maps = []
    for c in range(NCORES):
        b, r = divmod(c, 4)
        cs = slice(COLS * r, COLS * (r + 1))
        pb = np.zeros((64, 8, 128), np.float32)
        for bs in range(2):
            for h16 in range(16):
                ct, half = divmod(h16, 2)
                pb[32 * bs + h16, ct, 64 * half:64 * (half + 1)] = 1.0
        in_maps.append({
            "xqT": bf(q[b].T),
            "xkT": bf(k[b].T),
            "xvT": bf(v[b].T),
            "wq": bf(Wq[:, cs]),
            "wk": bf(Wk[:, cs]),
            "wv": bf(Wv[:, cs]),
            "wp": wp_bf,
            "maskT": bf(mask[b, 0].T.astype(np.float32)),
            "bp_rep": bp_rep,
            "pb_all": bf(pb.reshape(64, 1024)),
        })
    return in_maps


def assemble_out(results, N=N_FULL):
    out = np.empty((B, N, DIM), np.float32)
    for c in range(NCORES):
        for nh in range(N // 1024):
            for b in range(B):
                out[b, 1024 * nh + 128 * c:1024 * nh + 128 * (c + 1), :] = (
                    results[c]["out"][128 * (2 * nh + b):
                                      128 * (2 * nh + b + 1), :]
                )
    return out


_NC_CACHE = {}


def _get_nc():
    if "nc" not in _NC_CACHE:
        _NC_CACHE["nc"] = build_nc()
    return _NC_CACHE["nc"]


def kernel(q, k, v, mask, Wq, Wk, Wv, Wp, bp):
    from concourse.bass_utils import run_bass_kernel_spmd

    q, k, v = (np.asarray(a, np.float32) for a in (q, k, v))
    mask = np.asarray(mask)
    Wq, Wk, Wv, Wp, bp = (
        np.asarray(a, np.float32) for a in (Wq, Wk, Wv, Wp, bp)
    )
    nc = _get_nc()
    in_maps = make_in_maps(q, k, v, mask, Wq, Wk, Wv, Wp, bp)
    res = run_bass_kernel_spmd(nc, in_maps, core_ids=list(range(NCORES)))
    return assemble_out(res.results)


# revision 83
# speedup vs baseline: 1.4517x; 1.4517x over previous
"""Multi-head attention (B=2, N=2048, DIM=1024, H=16) on 8 Trainium2 NeuronCores.

Sharding: tensor-parallel by head within two quads (cores 0-3 -> batch 0,
cores 4-7 -> batch 1; quad rank r owns heads 4r..4r+3). Each core computes
Q/K/V projections for its 4 heads and masked-softmax attention. The output
projection is sharded over tokens ACROSS BATCHES: core c owns the 128-token
block [1024*nh + 128*c, +128) of BOTH batches for each query chunk nh, so
the 8-core head->token AllToAll carries no wasted bytes and splits into two
chunk collectives; the first overlaps the second chunk's attention compute.

Key engine assignment (vs the identity-matmul baseline):
- mask is applied multiplicatively on the vector engine (pm = exp(s)*mask01,
  both multiplies on DVE - the pool engine is 2x slower and its longer SBUF
  occupancy throttles the PE stream rate: ~620ns vs ~437ns per 512-col
  matmul), removing ~100us/core of PE identity-matmul time;
- the attn@V matmuls for tile t-1 are emitted after the score matmuls for
  tile t (software pipelining), so the in-order tensor queue never stalls
  on exp/mask of the current tile;
- softmax denominators come from a ones-column in V (row 64 of the attn@V
  accumulator) and ride RAW through the AllToAll as row 64 of each head
  block; one batched reciprocal per chunk + a PE broadcast (host-provided
  0/1 pattern, partition-aligned) normalizes after the collective;
- the scalar engine runs ONLY the exp activations (~140us/core); PSUM
  evictions run on the vector engine; phase-3 input DMAs are issued from
  both the sync and pool queues and are emitted right after each
  collective so they fire mid-attention.

Numerics: matmuls bf16 with fp32 PSUM accumulation (fp8 was tested and is
numerically dead here: query-side quantization noise is correlated across
keys, giving ~4% output error vs the 2% gate); exp without max-subtraction
(scores ~N(0,1) after scaling); denominators in bf16 (~0.4% relerr).
NOTE: the mask multiply must NOT be done in place (pu *= mask): the DVE
read-modify-write showed rare timing-dependent corruption (~7% of profiled
runs produced 7e-2 rel err); the separate pm tile is deliberate.
Measured end-to-end L2 relative error ~6.7e-3; HW exec ~350-385us
(baseline: 535us; run-to-run variance is DVFS throttling).
"""

import numpy as np
import ml_dtypes

import concourse.bass as bass
import concourse.mybir as mybir
import concourse.tile as tile

F32 = mybir.dt.float32
BF16 = mybir.dt.bfloat16
BF16_NP = ml_dtypes.bfloat16

B, DIM, H = 2, 1024, 16
N_FULL = 2048
HD = DIM // H          # 64
SCALE = HD ** -0.5     # 0.125
NCORES = 8
H_LOC = H // 4         # 4 heads per core
COLS = H_LOC * HD      # 256 local channels
KT_D = DIM // 128      # 8 contraction tiles over DIM
GROUPS = [list(range(NCORES))]


# ---------------------------------------------------------------------------
# Workaround: this walrus build rejects >2 sync waits on one instruction
# ("Too many sync wait commands" in setupSyncWait). The TileContext final
# drain aggregates one wait per logical processor; split it into a chain of
# single-wait drains.
# ---------------------------------------------------------------------------
def _patch_tile_drain():
    from bass_rust import ScopedClock

    if getattr(tile.TileContext, "_drain_patched", False):
        return

    def _drain_and_barrier(self, tick_clock, wait_clock):
        nc = self.nc
        drain_inst = nc.sync.drain()
        wait_clock.add_sem_waits(
            drain_inst.ins, ScopedClock({None: tick_clock.global_clock})
        )
        si = drain_inst.ins.sync_info
        if si is not None and len(si.on_wait) > 1:
            waits = list(si.on_wait)
            drain_inst.ins.sync_info = mybir.SyncInfo(
                on_wait=waits[:1], on_update=list(si.on_update)
            )
            for w in waits[1:]:
                d = nc.sync.drain()
                dsi = d.ins.sync_info
                upd = list(dsi.on_update) if dsi is not None else []
                d.ins.sync_info = mybir.SyncInfo(on_wait=[w], on_update=upd)

        nc.all_engine_barrier()
        assert self.sems is not None
        popped = nc._tile_sem_poison_stack.pop()
        assert popped is self._sem_poison
        nc.clear_and_free_semaphores(list(self.sems.allocated().values()))
        nc.all_engine_barrier()

    tile.TileContext._drain_and_barrier = _drain_and_barrier
    tile.TileContext._drain_patched = True


def _split_sync_waits(nc, maxw=1):
    """Walrus in this build rejects instructions carrying more than a couple
    of semaphore waits. Move excess waits onto injected same-engine NoOps
    immediately before the instruction (identical semantics: the engine
    blocks at the nop instead of at the instruction itself)."""
    n_split = 0
    for f in nc.m.functions:
        for bb in f.blocks:
            new_insts = []
            for ins in bb.instructions:
                si = ins.sync_info
                if si is not None and len(si.on_wait) > maxw:
                    waits = list(si.on_wait)
                    for i, w in enumerate(waits[maxw:]):
                        nop = mybir.InstNoOp(
                            name=f"{ins.name}-w{i}", ins=[], outs=[]
                        )
                        nop.engine = ins.engine
                        nop.sync_info = mybir.SyncInfo(
                            on_wait=[w], on_update=[]
                        )
                        new_insts.append(nop)
                    ins.sync_info = mybir.SyncInfo(
                        on_wait=waits[:maxw], on_update=list(si.on_update)
                    )
                    n_split += 1
                new_insts.append(ins)
            bb.instructions = new_insts
    return n_split


def build_nc(N=N_FULL, split_waits=True):
    """Build the per-core Bass program (same SPMD program for all 8 cores)."""
    _patch_tile_drain()
    assert N % 1024 == 0
    NSLICE = N // 4            # output rows per core
    MT = N // 128              # m-tiles over keys
    HS = 1024                  # attention query-chunk size
    NH = N // HS               # number of query chunks
    CT = DIM // 128            # contraction tiles for output projection

    nc = bass.Bass(trn_type="TRN2", num_devices=NCORES)

    xqT_e = nc.declare_dram_parameter("xqT", [DIM, N], BF16, isOutput=False)
    xkT_e = nc.declare_dram_parameter("xkT", [DIM, N], BF16, isOutput=False)
    xvT_e = nc.declare_dram_parameter("xvT", [DIM, N], BF16, isOutput=False)
    wq_e = nc.declare_dram_parameter("wq", [DIM, COLS], BF16, isOutput=False)
    wk_e = nc.declare_dram_parameter("wk", [DIM, COLS], BF16, isOutput=False)
    wv_e = nc.declare_dram_parameter("wv", [DIM, COLS], BF16, isOutput=False)
    wp_e = nc.declare_dram_parameter("wp", [DIM, DIM], BF16, isOutput=False)
    maskT_e = nc.declare_dram_parameter("maskT", [N, N], BF16, isOutput=False)
    bpr_e = nc.declare_dram_parameter("bp_rep", [128, DIM], F32, isOutput=False)
    pb_e = nc.declare_dram_parameter("pb_all", [64, CT * 128], BF16,
                                     isOutput=False)
    out_e = nc.declare_dram_parameter("out", [NSLICE, DIM], F32, isOutput=True)

    # per-chunk AllToAll buffers: chunk to dest j = [65 rows, (head g,
    # 128 tokens)]; row 64 carries the raw denominators. 1KB rows keep the
    # collective's DMA descriptors efficient.
    a2a_in = [nc.dram_tensor(f"a2a_in{i}", [NCORES * 65, H_LOC * 128], BF16)
              for i in range(NH)]
    a2a_out = [nc.dram_tensor(f"a2a_out{i}", [NCORES * 65, H_LOC * 128], BF16)
               for i in range(NH)]

    with tile.TileContext(nc) as tc:
        with (
            tc.tile_pool(name="cpool", bufs=1) as cpool,
            tc.tile_pool(name="xstream", bufs=5) as xpool,
            tc.tile_pool(name="mpool", bufs=18) as mpool,
            tc.tile_pool(name="pupool", bufs=6) as pupool,
            tc.tile_pool(name="pmpool", bufs=5) as pmpool,
            tc.tile_pool(name="p3pool", bufs=3) as p3pool,
            tc.tile_pool(name="opool", bufs=2) as opool,
            tc.tile_pool(name="ps", bufs=1, space="PSUM") as ps,
        ):
            # PSUM: four 2-bank (4KB/partition) tag slots shared by all
            # phases; static pool allocation = 8 banks.
            PST = ["S0", "S1", "V0", "V1"]

            # ---- long-lived SBUF tensors -------------------------------
            qt_sb = [cpool.tile([128, N], BF16, tag=f"qt{i}", name=f"qt{i}")
                     for i in range(2)]
            kt_sb = [cpool.tile([128, N], BF16, tag=f"kt{i}", name=f"kt{i}")
                     for i in range(2)]
            # V per m-tile: [m, head, 65]; cols 0..63 = V_head, col 64 = ones
            vt_sb = [cpool.tile([128, H_LOC, 65], BF16, tag=f"vt{t}",
                                name=f"vt{t}")
                     for t in range(MT)]
            # per-local-head x^T (unnormalized), row 64 = 1/denominator
            xt_sb = cpool.tile([65, H_LOC, N], BF16, tag="xth", name="xth")
            bpr_sb = cpool.tile([128, DIM], F32, tag="bpr", name="bpr")
            wq_sb = cpool.tile([128, KT_D, COLS], BF16, tag="wq", name="wq")
            wk_sb = cpool.tile([128, KT_D, COLS], BF16, tag="wk", name="wk")
            wv_sb = cpool.tile([128, KT_D, COLS], BF16, tag="wv", name="wv")
            xv_sb = cpool.tile([128, KT_D, N], BF16, tag="xv", name="xv")
            wp_sb = cpool.tile([128, CT, DIM], BF16, tag="wp", name="wp")
            # phase-3 denominator broadcast patterns: for batch section bs
            # at base partition 32*bs, row (32*bs + h16), slab ct, col p is
            # 1 iff h16 == 2*ct + p//64
            pb_sb = cpool.tile([64, CT, 128], BF16, tag="pb", name="pb")
            # gathered raw denominators, rows (32*bs + 4*src + g); unused
            # rows stay at the memset value
            rc_sb = [cpool.tile([64, 128], BF16, tag=f"rc{i}", name=f"rc{i}")
                     for i in range(NH)]

            # ---- phase 1: projections ----------------------------------
            # Q^T and K^T: [COLS, N] as two 128-row blocks; kt-outer with
            # one live [128, 1024] psum accumulator per (block, n-half).
            # DMA emission order keeps the startup critical path minimal:
            # only wq + the first xq tiles gate the first matmuls; all
            # later-phase constants are emitted after the Q loop starts.
            wq_v = wq_e[:].rearrange("(kt p) c -> p kt c", p=128)
            wk_v = wk_e[:].rearrange("(kt p) c -> p kt c", p=128)
            wv_v = wv_e[:].rearrange("(kt p) c -> p kt c", p=128)
            nc.sync.dma_start(wq_sb[:, 0:2, :], wq_v[:, 0:2, :])
            nc.sync.dma_start(wq_sb[:, 2:KT_D, :], wq_v[:, 2:KT_D, :])
            NH2 = N // 1024
            for w_sb, x_e, dst in (
                (wq_sb, xqT_e, qt_sb),
                (wk_sb, xkT_e, kt_sb),
            ):
                psums = [ps.tile([128, 1024], F32, tag=PST[cb * NH2 + n2],
                                 name="p1qk")
                         for cb in range(2) for n2 in range(NH2)]
                for kt in range(KT_D):
                    xt_t = xpool.tile([128, N], BF16, tag="xs", name="xs")
                    # two half-tile DMAs so the first matmuls start sooner
                    nc.sync.dma_start(
                        xt_t[:, 0:N // 2],
                        x_e[128 * kt:128 * (kt + 1), 0:N // 2],
                    )
                    nc.sync.dma_start(
                        xt_t[:, N // 2:N],
                        x_e[128 * kt:128 * (kt + 1), N // 2:N],
                    )
                    for cb in range(2):
                        for nch in range(N // 512):
                            n2, ch = divmod(nch, 2)
                            nc.tensor.matmul(
                                psums[cb * NH2 + n2][:, 512 * ch:512 * (ch + 1)],
                                w_sb[:, kt, 128 * cb:128 * (cb + 1)],
                                xt_t[:, 512 * nch:512 * (nch + 1)],
                                start=(kt == 0), stop=(kt == KT_D - 1),
                            )
                    if w_sb is wq_sb:
                        if kt == 0:
                            nc.sync.dma_start(wk_sb[:], wk_v)
                        elif kt == 1:
                            nc.sync.dma_start(wv_sb[:], wv_v)
                    else:
                        # stream xv in per-kt slices through the K loop so
                        # it never lumps against the x-tile stream
                        xv_v = xvT_e[:].rearrange("(kt p) n -> p kt n", p=128)
                        nc.sync.dma_start(xv_sb[:, kt, :], xv_v[:, kt, :])
                with nc.allow_low_precision(reason="bf16 Q/K"):
                    for cb in range(2):
                        for n2 in range(NH2):
                            nc.vector.tensor_copy(
                                dst[cb][:, 1024 * n2:1024 * (n2 + 1)],
                                psums[cb * NH2 + n2][:],
                            )

            # late-phase constants (emitted once the QK streams are rolling)
            nc.sync.dma_start(bpr_sb[:], bpr_e[:])
            nc.sync.dma_start(
                pb_sb[:], pb_e[:].rearrange("r (ct i) -> r ct i", ct=CT)
            )
            for i in range(NH):
                nc.gpsimd.memset(rc_sb[i][:], 1.0)
            for t in range(MT):
                nc.gpsimd.memset(vt_sb[t][:, :, 64:65], 1.0)

            # V in natural layout: out[m-tile, 4*HD] = xvT_kt^T @ wv_kt
            for t in range(MT):
                vps = ps.tile([128, 1024], F32, tag=PST[t % 2], name="p1v")
                for kt in range(KT_D):
                    nc.tensor.matmul(
                        vps[:, 0:COLS],
                        xv_sb[:, kt, 128 * t:128 * (t + 1)],
                        wv_sb[:, kt, :],
                        start=(kt == 0), stop=(kt == KT_D - 1),
                    )
                with nc.allow_low_precision(reason="bf16 V"):
                    nc.vector.tensor_copy(
                        vt_sb[t][:, :, 0:HD],
                        vps[:, 0:COLS].rearrange("p (h d) -> p h d", h=H_LOC),
                    )

            # ---- phase 2: attention ------------------------------------
            # Scores s = K_h^T Q_h per 128-key m-tile (head pairs packed in
            # 64-row PE groups); exp on the scalar engine; 0/1 mask applied
            # multiplicatively on the vector engine; attn@[V|ones]
            # accumulates x^T and the denominator row in PSUM.
            mask_t = {}
            p3_tiles = {}

            def load_mask(nh_, t_):
                if (nh_, t_) in mask_t:
                    return
                m = mpool.tile([128, HS], BF16, tag="m", name="m")
                nc.sync.dma_start(
                    m[:],
                    maskT_e[128 * t_:128 * (t_ + 1), HS * nh_:HS * (nh_ + 1)],
                )
                mask_t[(nh_, t_)] = m

            for t in range(4):
                load_mask(0, t)
            # wp is phase-3-only; emit its DMA after the mask prefetch
            wp_v = wp_e[:].rearrange("(ct p) c -> p ct c", p=128)
            nc.sync.dma_start(wp_sb[:], wp_v)

            for nh in range(NH):
                nsl = slice(HS * nh, HS * (nh + 1))
                for hp in range(2):
                    vo = [ps.tile([128, HS], F32, tag=PST[2 + h], name="vo")
                          for h in range(2)]

                    def av_mms(t_, pm_):
                        for h in range(2):
                            for ch in range(2):
                                csl = slice(512 * ch, 512 * (ch + 1))
                                nc.tensor.matmul(
                                    vo[h][0:65, csl],
                                    vt_sb[t_][:, 2 * hp + h, :],
                                    pm_[h][:, csl],
                                    start=(t_ == 0), stop=(t_ == MT - 1),
                                )

                    prev = None
                    for t in range(MT):
                        # prefetch masks: rest of this chunk during hp0,
                        # next chunk during hp1
                        if hp == 0 and t + 4 < MT:
                            load_mask(nh, t + 4)
                        elif hp == 1 and nh + 1 < NH:
                            load_mask(nh + 1, t)
                        s_ps = [ps.tile([128, HS], F32, tag=PST[h], name="s")
                                for h in range(2)]
                        for ch in range(2):
                            csl = slice(512 * ch, 512 * (ch + 1))
                            gsl = slice(HS * nh + 512 * ch,
                                        HS * nh + 512 * (ch + 1))
                            for h in range(2):
                                nc.tensor.matmul(
                                    s_ps[h][:, csl],
                                    kt_sb[hp][64 * h:64 * (h + 1),
                                              128 * t:128 * (t + 1)],
                                    qt_sb[hp][64 * h:64 * (h + 1), gsl],
                                    start=True, stop=True,
                                    tile_position=(64 * h, 0),
                                )
                        # software pipeline: the attn@V matmuls for t-1 are
                        # emitted AFTER the scores for t, so the tensor
                        # engine never waits on exp/mask of the current tile
                        if prev is not None:
                            av_mms(*prev)
                        pms = []
                        for h in range(2):
                            pu = pupool.tile([128, HS], BF16, tag="pu",
                                             name="pu")
                            nc.scalar.activation(
                                pu[:], s_ps[h][:],
                                mybir.ActivationFunctionType.Exp,
                                scale=float(SCALE),
                            )
                            pm = pmpool.tile([128, HS], BF16, tag="pm",
                                             name="pm")
                            with nc.allow_low_precision(reason="bf16 probs"):
                                nc.vector.tensor_mul(
                                    pm[:], pu[:], mask_t[(nh, t)][:]
                                )
                            pms.append(pm)
                        prev = (t, pms)
                    av_mms(*prev)
                    # evict x^T rows + raw denominator row (reciprocal
                    # happens post-AllToAll on small phase-3 tiles)
                    for h in range(2):
                        g = 2 * hp + h
                        with nc.allow_low_precision(reason="bf16 y/denom"):
                            nc.vector.tensor_copy(
                                xt_sb[0:64, g, nsl], vo[h][0:64, :]
                            )
                            nc.vector.tensor_copy(
                                xt_sb[64:65, g, nsl], vo[h][64:65, :]
                            )
                # AllToAll for this query chunk: head-split -> token-split
                # (dest core j owns tokens [1024*nh + 128*j, +128) of both
                # batches)
                a2a_in_v = a2a_in[nh][:].rearrange(
                    "(j p) c -> j p c", j=NCORES
                )
                for j in range(NCORES):
                    csl = slice(HS * nh + 128 * j, HS * nh + 128 * (j + 1))
                    nc.sync.dma_start(
                        a2a_in_v[j].rearrange("p (g i) -> p g i", g=H_LOC),
                        xt_sb[:, :, csl],
                    )
                nc.gpsimd.collective_compute(
                    "AllToAll",
                    mybir.AluOpType.bypass,
                    replica_groups=GROUPS,
                    ins=[a2a_in[nh][:]],
                    outs=[a2a_out[nh][:]],
                )
                # emit this chunk's phase-3 input DMAs NOW so the in-order
                # sync engine issues them as soon as the collective lands
                # (i.e. during the next chunk's attention compute)
                a2a_o_v = a2a_out[nh][:].rearrange(
                    "(s p) c -> s p c", s=NCORES
                )
                a2a_o_g = a2a_out[nh][:].rearrange(
                    "r (g i) -> (r g) i", g=H_LOC
                )
                # issue these from the (otherwise idle) pool engine queue
                # so they don't serialize behind the sync engine's DMAs
                for s in range(NCORES):
                    bs, sl = divmod(s, 4)
                    nc.gpsimd.dma_start(
                        rc_sb[nh][32 * bs + 4 * sl:32 * bs + 4 * sl + 4, :],
                        a2a_o_g[4 * (65 * s + 64):4 * (65 * s + 64) + 4, :],
                    )
                for ct in range(CT):
                    g0 = (2 * ct) % 4
                    for bs in range(2):
                        s = 4 * bs + (2 * ct) // 4
                        aa = p3pool.tile([128, 128], BF16, tag="aa",
                                         name="aa", bufs=16)
                        for i in range(2):
                            eng = nc.gpsimd if i == 0 else nc.sync
                            eng.dma_start(
                                aa[64 * i:64 * (i + 1), :],
                                a2a_o_v[s, 0:64,
                                        128 * (g0 + i):128 * (g0 + i + 1)],
                            )
                        p3_tiles[(nh, ct, bs)] = aa

            # ---- phase 3: normalize + output projection ----------------
            # Received layout: rows (src, 65), cols (g, 128 tok): head
            # 4*src+g has 64 x^T rows + a raw-denominator row. Two batch
            # sections (src 0-3 = batch 0, src 4-7 = batch 1); for each,
            # out[tok, :] = sum_ct (aa_ct / denom)^T @ Wp[ct] + bias.
            for nh in range(NH):
                # one batched reciprocal of all 32 gathered denominators
                rcf = p3pool.tile([64, 128], F32, tag="rcf", name="rcf")
                rcb = p3pool.tile([64, 128], BF16, tag="rcb", name="rcb")
                nc.vector.reciprocal(rcf[:], rc_sb[nh][:])
                with nc.allow_low_precision(reason="bf16 recip"):
                    nc.vector.tensor_copy(rcb[:], rcf[:])
                pj = [ps.tile([128, DIM], F32, tag=PST[bs], name=f"pj{bs}")
                      for bs in range(2)]
                for ct in range(CT):
                    for bs in range(2):
                        aa = p3_tiles[(nh, ct, bs)]
                        rr = ps.tile([128, 1024], F32, tag=PST[2 + bs],
                                     name="rr")
                        nc.tensor.matmul(
                            rr[:, 0:128],
                            pb_sb[32 * bs:32 * bs + 16, ct, :],
                            rcb[32 * bs:32 * bs + 16, :],
                            start=True, stop=True,
                        )
                        rrb = p3pool.tile([128, 128], BF16, tag="rrb",
                                          name="rrb")
                        xn = p3pool.tile([128, 128], BF16, tag="xn", name="xn")
                        with nc.allow_low_precision(reason="bf16 norm"):
                            nc.vector.tensor_copy(rrb[:], rr[:, 0:128])
                            nc.vector.tensor_mul(xn[:], aa[:], rrb[:])
                        for oc in range(2):
                            nc.tensor.matmul(
                                pj[bs][:, 512 * oc:512 * (oc + 1)],
                                xn[:],
                                wp_sb[:, ct, 512 * oc:512 * (oc + 1)],
                                start=(ct == 0), stop=(ct == CT - 1),
                            )
                for bs in range(2):
                    o_t = opool.tile([128, DIM], F32, tag="ot", name="ot")
                    nc.vector.tensor_add(o_t[:], pj[bs][:], bpr_sb[:])
                    blk = 2 * nh + bs
                    nc.sync.dma_start(
                        out_e[128 * blk:128 * (blk + 1), :], o_t[:]
                    )

    if split_waits:
        _split_sync_waits(nc)
    return nc


def make_in_maps(q, k, v, mask, Wq, Wk, Wv, Wp, bp, N=N_FULL):
    """Shard + pre-transpose + bf16-cast the full inputs for the 8 cores."""
    bf = lambda a: np.ascontiguousarray(a).astype(BF16_NP)
    bp_rep = np.ascontiguousarray(
        np.broadcast_to(bp.astype(np.float32), (128, DIM))
    )
    wp_bf = bf(Wp)
    in_maps = []
    for c in range(NCORES):
        b, r = divmod(c, 4)
        cs = slice(COLS * r, COLS * (r + 1))
        pb = np.zeros((64, 8, 128), np.float32)
        for bs in range(2):
            for h16 in range(16):
                ct, half = divmod(h16, 2)
                pb[32 * bs + h16, ct, 64 * half:64 * (half + 1)] = 1.0
        in_maps.append({
            "xqT": bf(q[b].T),
            "xkT": bf(k[b].T),
            "xvT": bf(v[b].T),
            "wq": bf(Wq[:, cs]),
            "wk": bf(Wk[:, cs]),
            "wv": bf(Wv[:, cs]),
            "wp": wp_bf,
            "maskT": bf(mask[b, 0].T.astype(np.float32)),
            "bp_rep": bp_rep,
            "pb_all": bf(pb.reshape(64, 1024)),
        })
    return in_maps


def assemble_out(results, N=N_FULL):
    out = np.empty((B, N, DIM), np.float32)
    for c in range(NCORES):
        for nh in range(N // 1024):
            for b in range(B):
                out[b, 1024 * nh + 128 * c:1024 * nh + 128 * (c + 1), :] = (
                    results[c]["out"][128 * (2 * nh + b):
                                      128 * (2 * nh + b + 1), :]
                )
    return out


_NC_CACHE = {}


def _get_nc():
    if "nc" not in _NC_CACHE:
        _NC_CACHE["nc"] = build_nc()
    return _NC_CACHE["nc"]


def kernel(q, k, v, mask, Wq, Wk, Wv, Wp, bp):
    from concourse.bass_utils import run_bass_kernel_spmd

    q, k, v = (np.asarray(a, np.float32) for a in (q, k, v))
    mask = np.asarray(mask)
    Wq, Wk, Wv, Wp, bp = (
        np.asarray(a, np.float32) for a in (Wq, Wk, Wv, Wp, bp)
    )
    nc = _get_nc()
    in_maps = make_in_maps(q, k, v, mask, Wq, Wk, Wv, Wp, bp)
    res = run_bass_kernel_spmd(nc, in_maps, core_ids=list(range(NCORES)))
    return assemble_out(res.results)
